# revision 1
# baseline (speedup 1.0000x reference)
"""Causal multi-head attention on 8 Trainium2 NeuronCores.

Problem: x[8,1024,768], 12 heads of d_head=64, causal softmax attention,
output projection. Sharding: data-parallel over batch (8 batch elements ==
8 cores), no collectives.

Per-core layout strategy (zero on-device transposes):
  - host passes xT [768,1024] (d_model on partitions)
  - qT/kT [768,1024] head-major rows  (d_head on partitions, seq on free)
  - scoresT[k, q] = kT_blk.T @ qT     (k on partitions, q on free)
  - v_aug [1024, 12*128]: per head 64 v columns + 64 ones columns; the AV
    matmul zT_psum = v_aug_blk.T @ exp(scoresT) then yields the softmax
    denominator (replicated) in psum partitions 64..127 for free
  - normalize with DVE reciprocal + tensor_mul (one PSUM operand)
  - out = zT.T @ W_O accumulated over head-pair chunks
Causal structure: only the lower-triangular (k <= q) blocks are computed;
the diagonal 128x128 block is zeroed above the diagonal post-exp via
affine_select. exp is computed without max subtraction (scores are O(1)
by construction, exp(-1e5) == 0 in fp32 matches the reference's masking).
Matmul dtypes: fp32r (full PE stream rate at free-dim >= 256, fp32 PSUM
accumulate) for everything on the value path; the Q/K projections run with
bf16 operands — their self-loading weight phase is then hidden behind the
previous matmul (fp32r self-loading matmuls pay an un-overlapped ~107 ns
LDWEIGHTS stall each), and the bf16 quantization only perturbs the softmax
scores.  Measured end-to-end: rel err 6.0e-4 vs the fp32 reference,
~195-200 us on-device per invocation (8 cores in parallel).

Env switches (default off; A/B experiments):
  BASS_BF16=1    all matmul operands bf16 (~172 us, rel err 3.0e-3)
  BASS_SPLITK=1  split K=128 matmuls into row-group-interleaved K=64 halves
                 (crashes on hardware: concurrent same-bank PSUM accumulation
                 is not legal; kept for documentation)
"""

from contextlib import ExitStack

import numpy as np

import concourse.mybir as mybir
import concourse.tile as tile
from concourse import bacc, bass_utils

F32 = mybir.dt.float32
FR = mybir.dt.float32r
BF = mybir.dt.bfloat16
import os
# BASS_BF16=1: all matmul operands in bf16 (mixing bf16/fp32r is illegal on
# TRN2, so it is all-or-nothing).  PSUM accumulation stays fp32 either way.
BF16 = os.environ.get("BASS_BF16", "0") == "1"
MDT = BF if BF16 else FR           # dtype of every matmul operand tile
# BASS_SPLITK=1: emit every K=128 matmul as two interleaved K=64 matmuls on
# opposite PE row-groups.  The halves run concurrently (same stream time) and
# each half's self-loading fp32r weight load overlaps the other half's stream,
# hiding the per-matmul LDWEIGHTS stall.
SPLITK = os.environ.get("BASS_SPLITK", "0") == "1"
# BASS_VSPLIT=1: V-projection matmuls as two K=64 row-group halves into two
# separate PSUM banks (legal, unlike same-bank), merged by an extra DVE add.
# Alternating row groups lets each half's self-loading fp32r weight load
# overlap the other half's stream, hiding the ~107ns/MM LDWEIGHTS stall.
VSPLIT = os.environ.get("BASS_VSPLIT", "0") == "1"

S = 1024        # seq len
D = 768         # d_model
H = 12          # heads
DH = 64         # d_head
P = 128         # partitions
KC = D // P     # 6 k-chunks of d_model
SB = S // P     # 8 seq blocks
PAIRS = H // 2  # 6 head pairs
VW = 2 * DH     # 128: v cols + ones cols per head in v_aug
N_CORES = 8


def fr(ap):
    return ap


def mm128(nc, ps, lhsT, rhs, start, stop):
    """K=128 matmul, optionally split into two row-group-interleaved K=64
    matmuls (see SPLITK above)."""
    if not SPLITK:
        nc.tensor.matmul(ps, lhsT=lhsT, rhs=rhs, start=start, stop=stop)
        return
    nc.tensor.matmul(ps, lhsT=lhsT[0:DH, :], rhs=rhs[0:DH, :],
                     start=start, stop=False)
    nc.tensor.matmul(ps, lhsT=lhsT[DH:P, :], rhs=rhs[DH:P, :],
                     start=False, stop=stop)


def attention_kernel(tc, out_ap, ins):
    nc = tc.nc
    with ExitStack() as ctx:
        cpool = ctx.enter_context(tc.tile_pool(name="consts", bufs=1))
        big = ctx.enter_context(tc.tile_pool(name="big", bufs=1))
        exp_pool = ctx.enter_context(tc.tile_pool(name="exp", bufs=4))
        rec_pool = ctx.enter_context(tc.tile_pool(name="rec", bufs=2))
        out_pool = ctx.enter_context(tc.tile_pool(name="outb", bufs=3))
        pp = ctx.enter_context(tc.tile_pool(name="ps", bufs=4, space="PSUM"))
        ppz = ctx.enter_context(tc.tile_pool(name="psz", bufs=4, space="PSUM"))

        # ---- big persistent tiles.  zT reuses xT's storage: xT is fully
        # consumed by the projections before any zT column is written (the
        # scheduler enforces the WAR ordering via subtile deps).
        xT = big.tile([P, KC * S], MDT, tag="xT")          # xT[kc]: cols kc*1024
        qT = big.tile([P, PAIRS * S], MDT, tag="qT")       # pair p: cols p*1024+q
        kT = big.tile([P, PAIRS * S], MDT, tag="kT")
        vaug = big.tile([P, SB * H * VW], MDT, tag="vaug")  # blk j: j*1536 + n*128
        zT = xT
        xv = xT
        # bf16 copy of x for the Q/K projections: those matmuls run all-bf16
        # (both operands), which restores the hidden LDWEIGHTS path (fp32r
        # self-loading matmuls pay an un-overlapped ~107ns weight load per
        # matmul).  q/k only feed the softmax scores, so the bf16
        # quantization contributes well under 1e-3 to the output.
        xb = big.tile([P, KC * S], BF, tag="xb")

        # xb first on the ACT sequencer (feeds the first matmuls; half the
        # bytes of xT); fp32r xT for the V-projection goes via gpsimd/SWDGE.
        for kc in range(KC):
            nc.scalar.dma_start(xb[:, kc * S:(kc + 1) * S],
                                ins["xb"][kc * P:(kc + 1) * P, :])
        for kc in range(KC):
            nc.gpsimd.dma_start(xT[:, kc * S:(kc + 1) * S],
                                ins["xT"][kc * P:(kc + 1) * P, :])

        # ---- constants / small tiles (issued on the Pool sequencer)
        bqt = cpool.tile([P, PAIRS], F32, tag="bqt")   # b_Q per head-major row
        bkt = cpool.tile([P, PAIRS], F32, tag="bkt")
        bvb = cpool.tile([P, D], F32, tag="bvb")       # b_V broadcast to 128 rows
        nc.gpsimd.dma_start(bqt[:], ins["bqt"].rearrange("(c p) x -> p (c x)", p=P))
        nc.gpsimd.dma_start(bkt[:], ins["bkt"].rearrange("(c p) x -> p (c x)", p=P))
        nc.gpsimd.dma_start(bvb[:], ins["bvb"][:])

        # ones columns of v_aug (cols 64..127 of each head block), filled by
        # DVE copies (f32 -> fp32r rounding) from a memset source tile
        ones_f = cpool.tile([P, D], F32, tag="ones_f")
        nc.gpsimd.memset(ones_f[:], 1.0)
        ones_f3 = ones_f[:].rearrange("p (x c) -> p x c", c=DH)  # [128, 12, 64]
        for s in range(SB):
            blk = vaug[:, s * H * VW:(s + 1) * H * VW]
            va3 = blk.rearrange("p (x c) -> p x c", c=VW)
            nc.vector.tensor_copy(va3[:, :, DH:VW], ones_f3)

        # per-pair column slices of W_Q/W_K, streamed: [768, 128] -> [128, 6*128]
        wq_r = ins["wq"].rearrange("(c p) n -> p c n", p=P)
        wk_r = ins["wk"].rearrange("(c p) n -> p c n", p=P)

        with tc.tile_pool(name="wa", bufs=1) as wa:
            wv = wa.tile([P, KC * D], MDT, tag="wv")
            for kc in range(KC):
                nc.gpsimd.dma_start(wv[:, kc * D:(kc + 1) * D],
                                    ins["wv"][kc * P:(kc + 1) * P, :])

            # ---- Q/K projections: qT[pair rows, q] = W[:, pair].T @ xT
            for p in range(PAIRS):
                wqp = wa.tile([P, KC * P], BF, tag="wqp", bufs=2,
                              name=f"wqp_{p}")
                wkp = wa.tile([P, KC * P], BF, tag="wkp", bufs=2,
                              name=f"wkp_{p}")
                nc.sync.dma_start(
                    wqp[:].rearrange("q (c n) -> q c n", n=P),
                    wq_r[:, :, p * P:(p + 1) * P])
                nc.scalar.dma_start(
                    wkp[:].rearrange("q (c n) -> q c n", n=P),
                    wk_r[:, :, p * P:(p + 1) * P])
                for half in range(2):
                    for w_sb, b_sb, dst in ((wqp, bqt, qT), (wkp, bkt, kT)):
                        ps = pp.tile([P, 512], F32, tag="ps")
                        for kc in range(KC):
                            nc.tensor.matmul(
                                ps[:],
                                lhsT=w_sb[:, kc * P:(kc + 1) * P],
                                rhs=xb[:, kc * S + half * 512: kc * S + (half + 1) * 512],
                                start=(kc == 0), stop=(kc == KC - 1))
                        # bias is per output partition (head-major row):
                        # fold it into the psum->sbuf copy
                        nc.vector.tensor_scalar_add(
                            dst[:, p * S + half * 512: p * S + (half + 1) * 512],
                            ps[:], b_sb[:, p:p + 1])

            # ---- V projection into v_aug (strided per-head placement)
            for s in range(SB):
                for cb, n_cols in ((0, 512), (1, 256)):
                    nh = n_cols // DH  # heads in this column block
                    base = s * H * VW + cb * 8 * VW
                    dst3 = vaug[:, base: base + nh * VW].rearrange(
                        "p (n c) -> p n c", c=VW)[:, :, 0:DH]
                    bv3 = bvb[:, cb * 512: cb * 512 + n_cols].rearrange(
                        "p (n c) -> p n c", c=DH)
                    if VSPLIT:
                        psa = pp.tile([P, n_cols], F32, tag="ps",
                                      name=f"vps_a_{s}_{cb}")
                        psb = pp.tile([P, n_cols], F32, tag="ps",
                                      name=f"vps_b_{s}_{cb}")
                        for kc in range(KC):
                            for hps, r0 in ((psa, 0), (psb, DH)):
                                nc.tensor.matmul(
                                    hps[:],
                                    lhsT=fr(xv[r0:r0 + DH, kc * S + s * P: kc * S + (s + 1) * P]),
                                    rhs=fr(wv[r0:r0 + DH, kc * D + cb * 512: kc * D + cb * 512 + n_cols]),
                                    start=(kc == 0), stop=(kc == KC - 1))
                        a3 = psa[:, 0:n_cols].rearrange("p (n c) -> p n c", c=DH)
                        b3 = psb[:, 0:n_cols].rearrange("p (n c) -> p n c", c=DH)
                        nc.vector.tensor_add(dst3, a3, bv3)
                        nc.vector.tensor_add(dst3, b3, dst3)
                    else:
                        ps = pp.tile([P, n_cols], F32, tag="ps")
                        for kc in range(KC):
                            mm128(
                                nc, ps[:],
                                fr(xv[:, kc * S + s * P: kc * S + (s + 1) * P]),
                                fr(wv[:, kc * D + cb * 512: kc * D + cb * 512 + n_cols]),
                                (kc == 0), (kc == KC - 1))
                        src3 = ps[:, 0:n_cols].rearrange("p (n c) -> p n c", c=DH)
                        nc.vector.tensor_add(dst3, src3, bv3)

        # ---- attention per head pair
        # score_pieces(j): q-ranges for the score matmuls / exp, chosen >=256
        # wide where possible (fp32r matmuls run 4x slower below N=256).
        # av_slices(j): q-ranges of AV matmuls, aligned to the two z psum
        # banks (the AV rhs reads the exp SBUF tile, so the boundaries are
        # independent of score_pieces).
        # qs_eff: the q-start of the computed strip for kblock j.  For j=3 and
        # j=7 the strip is widened 128 into the masked region so that every
        # score/AV matmul has free dim >= 256 (fp32r runs 4x slower below
        # that); the widened part is zeroed by the affine_select, so the AV
        # accumulation just adds zeros there.
        def qs_eff(j):
            return (j - 1) * P if j in (3, 7) else j * P

        def score_pieces(j):
            qs, L, out = qs_eff(j), S - qs_eff(j), []
            while L > 0:
                w = 512 if L >= 768 else (L if L <= 512 else L - 256)
                out.append((qs, qs + w))
                qs += w
                L -= w
            return out

        def av_slices(j):
            qs = qs_eff(j)
            sl = []
            if qs < 512:
                sl.append((qs, 512, 0))
            sl.append((max(qs, 512), 1024, 1))
            return sl

        for p in range(PAIRS):
            zps = [[None, None], [None, None]]  # [o][chunk]
            for o in range(2):
                for c in range(2):
                    zps[o][c] = ppz.tile([P, 512], F32, tag="psz",
                                         name=f"zps_{p}_{o}_{c}")
            for j in range(SB):
                q0 = qs_eff(j)
                expt = [None, None]
                for o in range(2):
                    expt[o] = exp_pool.tile([P, S - q0], MDT, tag="exp",
                                            name=f"exp_{p}_{j}_{o}")
                # scores + exp (interleave heads for PE row-group concurrency)
                for (qs, qe) in score_pieces(j):
                    sps = [None, None]
                    for o in range(2):
                        sps[o] = pp.tile([P, qe - qs], F32, tag="ps",
                                         name=f"sps_{p}_{j}_{qs}_{o}")
                        nc.tensor.matmul(
                            sps[o][:],
                            lhsT=fr(kT[o * DH:(o + 1) * DH, p * S + j * P: p * S + (j + 1) * P]),
                            rhs=fr(qT[o * DH:(o + 1) * DH, p * S + qs: p * S + qe]),
                            start=True, stop=True)
                    for o in range(2):
                        nc.scalar.activation(
                            expt[o][:, qs - q0: qe - q0], sps[o][:],
                            mybir.ActivationFunctionType.Exp, scale=0.125)
                # zero where q < k over the leading cols (diagonal block plus
                # any widened pre-diagonal region): keep iff
                # (q0 - j*128) + col - partition >= 0
                wz = j * P + P - q0
                for o in range(2):
                    nc.gpsimd.affine_select(
                        out=expt[o][:, 0:wz], in_=expt[o][:, 0:wz],
                        compare_op=mybir.AluOpType.is_ge,
                        fill=0.0, base=q0 - j * P,
                        pattern=[[1, wz]], channel_multiplier=-1)
                # AV accumulation (+ denominator in partitions 64..127)
                for o in range(2):
                    n = 2 * p + o
                    for (qs, qe, c) in av_slices(j):
                        mm128(
                            nc, zps[o][c][:, qs - c * 512: qe - c * 512],
                            fr(vaug[:, j * H * VW + n * VW: j * H * VW + (n + 1) * VW]),
                            fr(expt[o][:, qs - q0: qe - q0]),
                            (j == 0),
                            (j == 3 if c == 0 else j == 7))
                # chunk 0 finishes at j==3: normalize early to free the bank
                if j == 3 or j == 7:
                    c = 0 if j == 3 else 1
                    for o in range(2):
                        n = 2 * p + o
                        rec = rec_pool.tile([DH, 512], F32, tag="rec")
                        nc.vector.reciprocal(rec[:], zps[o][c][DH:P, 0:512])
                        nc.vector.tensor_mul(
                            zT[o * DH:(o + 1) * DH, p * S + c * 512: p * S + (c + 1) * 512],
                            zps[o][c][0:DH, 0:512], rec[:])

        # ---- output projection: out[s*128.., m] = zT.T @ W_O + b_O
        with tc.tile_pool(name="wc", bufs=1) as wc:
            wo = wc.tile([P, KC * D], MDT, tag="wo")
            bob = wc.tile([P, D], F32, tag="bob")      # b_O broadcast to 128 rows
            nc.gpsimd.dma_start(bob[:], ins["bob"][:])
            for kc in range(KC):
                nc.sync.dma_start(wo[:, kc * D:(kc + 1) * D],
                                  ins["wo"][kc * P:(kc + 1) * P, :])
            for s in range(SB):
                outb = out_pool.tile([P, D], F32, tag="outb")
                for cb, n_cols in ((0, 512), (1, 256)):
                    ps = pp.tile([P, n_cols], F32, tag="ps")
                    for p in range(PAIRS):
                        mm128(
                            nc, ps[:],
                            fr(zT[:, p * S + s * P: p * S + (s + 1) * P]),
                            fr(wo[:, p * D + cb * 512: p * D + cb * 512 + n_cols]),
                            (p == 0), (p == PAIRS - 1))
                    nc.vector.tensor_add(outb[:, cb * 512: cb * 512 + n_cols],
                                         ps[:],
                                         bob[:, cb * 512: cb * 512 + n_cols])
                    nc.sync.dma_start(
                        out_ap[s * P:(s + 1) * P, cb * 512: cb * 512 + n_cols],
                        outb[:, cb * 512: cb * 512 + n_cols])


_CACHED = {}


def build_program(reps=1):
    if reps in _CACHED:
        return _CACHED[reps]
    nc = bacc.Bacc("TRN2", target_bir_lowering=False, debug=False)
    ins = {
        "xT": nc.dram_tensor("xT", [D, S], MDT, kind="ExternalInput").ap(),
        "xb": nc.dram_tensor("xb", [D, S], BF, kind="ExternalInput").ap(),
        "wq": nc.dram_tensor("wq", [D, D], BF, kind="ExternalInput").ap(),
        "wk": nc.dram_tensor("wk", [D, D], BF, kind="ExternalInput").ap(),
        "wv": nc.dram_tensor("wv", [D, D], MDT, kind="ExternalInput").ap(),
        "wo": nc.dram_tensor("wo", [D, D], MDT, kind="ExternalInput").ap(),
        "bqt": nc.dram_tensor("bqt", [D, 1], F32, kind="ExternalInput").ap(),
        "bkt": nc.dram_tensor("bkt", [D, 1], F32, kind="ExternalInput").ap(),
        "bvb": nc.dram_tensor("bvb", [P, D], F32, kind="ExternalInput").ap(),
        "bob": nc.dram_tensor("bob", [P, D], F32, kind="ExternalInput").ap(),
    }
    out = nc.dram_tensor("out", [S, D], F32, kind="ExternalOutput").ap()
    with tile.TileContext(nc) as tc:
        for _ in range(reps):
            attention_kernel(tc, out, ins)
    nc.compile()
    _CACHED[reps] = nc
    return nc


def make_in_maps(normalized_resid_pre, W_Q, W_K, W_V, W_O, b_Q, b_K, b_V, b_O):
    x = np.asarray(normalized_resid_pre, np.float32)
    import ml_dtypes
    bf = ml_dtypes.bfloat16
    wdt = bf if BF16 else np.float32
    wq_m = np.ascontiguousarray(
        np.asarray(W_Q, np.float32).transpose(1, 0, 2).reshape(D, D).astype(bf))
    wk_m = np.ascontiguousarray(
        np.asarray(W_K, np.float32).transpose(1, 0, 2).reshape(D, D).astype(bf))
    wv_m = np.ascontiguousarray(
        np.asarray(W_V, np.float32).transpose(1, 0, 2).reshape(D, D).astype(wdt))
    wo_m = np.ascontiguousarray(
        np.asarray(W_O, np.float32).reshape(D, D).astype(wdt))
    bq_m = np.asarray(b_Q, np.float32).reshape(D, 1)
    bk_m = np.asarray(b_K, np.float32).reshape(D, 1)
    bv_m = np.ascontiguousarray(np.broadcast_to(
        np.asarray(b_V, np.float32).reshape(1, D), (P, D)))
    bo_m = np.ascontiguousarray(np.broadcast_to(
        np.asarray(b_O, np.float32).reshape(1, D), (P, D)))
    in_maps = []
    for b in range(N_CORES):
        in_maps.append({
            "xT": np.ascontiguousarray(x[b].T).astype(wdt),
            "xb": np.ascontiguousarray(x[b].T).astype(bf),
            "wq": wq_m, "wk": wk_m, "wv": wv_m, "wo": wo_m,
            "bqt": bq_m, "bkt": bk_m, "bvb": bv_m, "bob": bo_m,
        })
    return in_maps


def kernel(**inputs):
    nc = build_program()
    in_maps = make_in_maps(**inputs)
    res = bass_utils.run_bass_kernel_spmd(nc, in_maps, list(range(N_CORES)))
    return np.stack([r["out"] for r in res.results])



# revision 5
# speedup vs baseline: 1.3123x; 1.3123x over previous
"""Causal multi-head attention on 8 Trainium2 NeuronCores.

Problem: x[8,1024,768], 12 heads of d_head=64, causal softmax attention,
output projection. Sharding: data-parallel over batch (8 batch elements ==
8 cores), no collectives.

Per-core layout (zero on-device transposes):
  - host passes x.T [768,1024] bf16 (d_model on partitions)
  - qT/kT [768,1024] head-major rows  (d_head on partitions, seq on free)
  - scoresT[k, q] = kT_blk.T @ qT     (k on partitions, q on free); the two
    heads of a pair run as K=64 matmuls on opposite PE row-group halves
    (auto-derived tile_position) so they can stream concurrently on HW
  - v_aug [1024, 12*128]: per head 64 v columns + 64 ones columns; the AV
    matmul zT_psum = v_aug_blk.T @ exp(scoresT) then yields the softmax
    denominator (replicated) in psum partitions 64..127 for free
  - normalize with DVE reciprocal + tensor_mul (one PSUM operand)
  - out = zT.T @ W_O accumulated over head-pair chunks
Causal structure: only lower-triangular (k <= q) 128-blocks are computed;
the diagonal block is zeroed above the diagonal post-exp via affine_select.
exp runs without max subtraction (scores are O(1) by construction;
exp(-1e5) == 0 in fp32 matches the reference masking).

Schedule: the kernel is engine-balanced rather than phase-sequential.  The
exp traffic (ACT) is the co-bottleneck of the attention loop (~58us vs
~46us of score+AV PE work), while the Q/K/V projections are pure PE work
with ACT idle.  So the projections for pair p+1 are emitted interleaved
into the attention loop of pair p (one 6-matmul projection group per
(chunk, k-block) iteration), and the first half of the output projection
is interleaved into the last pair's attention.  Attention runs q-chunk
outer (two 512-wide q chunks), which needs only 2 PSUM banks for the z
accumulators and leaves 4 for double-buffered scores + 2 for the
projection groups (8 total).  All matmul operands are bf16 (FWL hides the
LDWEIGHTS phase; fp32r stalls ~107ns per matmul on the weight load); PSUM
accumulation is fp32.  End-to-end rel err ~3e-3 vs the fp32 reference
(tolerance 2e-2).

Back-to-back invocations pipeline: x / W_V / first W_Q/W_K pairs / bias
tiles are double-buffered, and each rep issues the next rep's input DMAs
mid-flight on the SP queue (which is idle mid-rep; the ACT/Pool queues
are head-of-line blocked by exp/mask work until the rep ends), so the
next rep's projections start the moment the PE drains.
"""

from contextlib import ExitStack

import numpy as np

import concourse.mybir as mybir
import concourse.tile as tile
from concourse import bacc, bass_utils

F32 = mybir.dt.float32
BF = mybir.dt.bfloat16
MDT = BF

S = 1024        # seq len
D = 768         # d_model
H = 12          # heads
DH = 64         # d_head
P = 128         # partitions
KC = D // P     # 6 k-chunks of d_model
SB = S // P     # 8 seq blocks
PAIRS = H // 2  # 6 head pairs
VW = 2 * DH     # 128: v cols + ones cols per head in v_aug
N_CORES = 8


def make_pools(tc, ctx):
    return {
        "consts": ctx.enter_context(tc.tile_pool(name="consts", bufs=2)),
        "xw": ctx.enter_context(tc.tile_pool(name="xw", bufs=2)),
        "big": ctx.enter_context(tc.tile_pool(name="big", bufs=1)),
        "wqk": ctx.enter_context(tc.tile_pool(name="wqk", bufs=4)),
        "wo": ctx.enter_context(tc.tile_pool(name="wo", bufs=2)),
        "exp": ctx.enter_context(tc.tile_pool(name="exp", bufs=6)),
        "rec": ctx.enter_context(tc.tile_pool(name="rec", bufs=2)),
        "outb": ctx.enter_context(tc.tile_pool(name="outb", bufs=3)),
        "pp": ctx.enter_context(tc.tile_pool(name="ps", bufs=4, space="PSUM")),
        "ppz": ctx.enter_context(tc.tile_pool(name="psz", bufs=2, space="PSUM")),
        "ppj": ctx.enter_context(tc.tile_pool(name="psj", bufs=2, space="PSUM")),
    }


def issue_inputs(tc, pools, ins, first):
    """Allocate the double-buffered input tiles for one rep and issue their
    DMAs.  At kernel start (`first`) the transfers spread across the idle
    ACT/Pool/SP queues; mid-rep prefetch uses only the SP queue (the others
    are head-of-line blocked by the current rep's work)."""
    nc = tc.nc
    st = {}
    st["xt"] = pools["xw"].tile([P, KC * S], MDT, tag="xt", name="xt")
    st["wv"] = pools["xw"].tile([P, KC * D], MDT, tag="wv", name="wv")
    st["bqt"] = pools["consts"].tile([P, PAIRS], F32, tag="bqt", name="bqt")
    st["bkt"] = pools["consts"].tile([P, PAIRS], F32, tag="bkt", name="bkt")
    st["bvb"] = pools["consts"].tile([P, D], F32, tag="bvb", name="bvb")
    st["wq_t"] = {}

    def wdma(p):
        wqp = pools["wqk"].tile([P, KC * P], BF, tag="wqp", name=f"wqp_{p}")
        wkp = pools["wqk"].tile([P, KC * P], BF, tag="wkp", name=f"wkp_{p}")
        nc.sync.dma_start(wqp[:], ins["wq"][p * P:(p + 1) * P, :])
        nc.sync.dma_start(wkp[:], ins["wk"][p * P:(p + 1) * P, :])
        st["wq_t"][p] = (wqp, wkp)

    st["wdma"] = wdma
    wdma(0)
    wdma(1)
    # x in column-halves (all kc of half 0 first: the first projection
    # groups only need q < 512)
    xq = [nc.scalar, nc.gpsimd] if first else [nc.sync, nc.sync]
    for half in range(2):
        for kc in range(KC):
            xq[kc % 2].dma_start(
                st["xt"][:, kc * S + half * 512: kc * S + (half + 1) * 512],
                ins["xT"][kc * P:(kc + 1) * P, half * 512:(half + 1) * 512])
    cq = nc.gpsimd if first else nc.sync
    cq.dma_start(st["bqt"][:], ins["bqt"].rearrange("(c p) x -> p (c x)", p=P))
    cq.dma_start(st["bkt"][:], ins["bkt"].rearrange("(c p) x -> p (c x)", p=P))
    cq.dma_start(st["bvb"][:], ins["bvb"][:])
    for kc in range(KC):
        cq.dma_start(st["wv"][:, kc * D:(kc + 1) * D],
                     ins["wv"][kc * P:(kc + 1) * P, :])
    return st


def attention_kernel(tc, out_ap, ins, pools, st, prefetch):
    nc = tc.nc
    xt, wv, wq_t = st["xt"], st["wv"], st["wq_t"]
    bqt, bkt, bvb = st["bqt"], st["bkt"], st["bvb"]
    big = pools["big"]

    qT = big.tile([P, PAIRS * S], MDT, tag="qT")       # pair p: cols p*1024+q
    kT = big.tile([P, PAIRS * S], MDT, tag="kT")
    zT = big.tile([P, PAIRS * S], MDT, tag="zT")
    vaug = big.tile([P, SB * H * VW], MDT, tag="vaug")  # blk j: j*1536 + n*128

    # ones columns of v_aug (cols 64..127 of each head block)
    ones_f = pools["consts"].tile([P, D], F32, tag="ones_f")
    nc.gpsimd.memset(ones_f[:], 1.0)
    ones_f3 = ones_f[:].rearrange("p (x c) -> p x c", c=DH)  # [128, 12, 64]
    for s in range(SB):
        blk = vaug[:, s * H * VW:(s + 1) * H * VW]
        va3 = blk.rearrange("p (x c) -> p x c", c=VW)
        nc.vector.tensor_copy(va3[:, :, DH:VW], ones_f3)

    wo = pools["wo"].tile([P, KC * D], MDT, tag="wo")
    bob = pools["wo"].tile([P, D], F32, tag="bob")  # b_O broadcast to 128 rows

    def emit_wo_dma():
        nc.gpsimd.dma_start(bob[:], ins["bob"][:])
        for kc in range(KC):
            nc.sync.dma_start(wo[:, kc * D:(kc + 1) * D],
                              ins["wo"][kc * P:(kc + 1) * P, :])

    # ---- projection groups (6 accumulating matmuls + one DVE drain)
    def emit_qk_group(p, half, which):
        wqp, wkp = wq_t[p]
        w_sb = wqp if which == 0 else wkp
        b_sb = bqt if which == 0 else bkt
        dst = qT if which == 0 else kT
        ps = pools["ppj"].tile([P, 512], F32, tag="psj")
        for kc in range(KC):
            nc.tensor.matmul(
                ps[:],
                lhsT=w_sb[:, kc * P:(kc + 1) * P],
                rhs=xt[:, kc * S + half * 512: kc * S + (half + 1) * 512],
                start=(kc == 0), stop=(kc == KC - 1))
        nc.vector.tensor_scalar_add(
            dst[:, p * S + half * 512: p * S + (half + 1) * 512],
            ps[:], b_sb[:, p:p + 1])

    def emit_v_group(p, s):
        # v for the two heads of pair p at seq block s: psum [128, 128]
        ps = pools["ppj"].tile([P, VW], F32, tag="psj", name=f"vps_{p}_{s}")
        for kc in range(KC):
            nc.tensor.matmul(
                ps[:],
                lhsT=xt[:, kc * S + s * P: kc * S + (s + 1) * P],
                rhs=wv[:, kc * D + p * VW: kc * D + (p + 1) * VW],
                start=(kc == 0), stop=(kc == KC - 1))
        base = s * H * VW + 2 * p * VW
        dst3 = vaug[:, base: base + 2 * VW].rearrange(
            "p (n c) -> p n c", c=VW)[:, :, 0:DH]
        src3 = ps[:].rearrange("p (n c) -> p n c", c=DH)
        bv3 = bvb[:, p * VW: (p + 1) * VW].rearrange("p (n c) -> p n c", c=DH)
        nc.vector.tensor_add(dst3, src3, bv3)

    def proj_groups(p):
        gs = []
        for half in range(2):
            for which in range(2):
                gs.append(lambda p=p, h=half, w=which: emit_qk_group(p, h, w))
        for s in range(SB):
            gs.append(lambda p=p, s=s: emit_v_group(p, s))
        return gs

    out_tiles = {}

    def outproj_group(s, cb):
        n_cols = 512 if cb == 0 else 256
        outb = out_tiles.get(s)
        if outb is None:
            outb = out_tiles[s] = pools["outb"].tile([P, D], F32, tag="outb",
                                                     name=f"outb_{s}")
        ps = pools["ppj"].tile([P, n_cols], F32, tag="psj", name=f"ops_{s}_{cb}")
        for p in range(PAIRS):
            nc.tensor.matmul(
                ps[:],
                lhsT=zT[:, p * S + s * P: p * S + (s + 1) * P],
                rhs=wo[:, p * D + cb * 512: p * D + cb * 512 + n_cols],
                start=(p == 0), stop=(p == PAIRS - 1))
        nc.vector.tensor_add(outb[:, cb * 512: cb * 512 + n_cols],
                             ps[:],
                             bob[:, cb * 512: cb * 512 + n_cols])
        nc.sync.dma_start(
            out_ap[s * P:(s + 1) * P, cb * 512: cb * 512 + n_cols],
            outb[:, cb * 512: cb * 512 + n_cols])

    # ---- attention for pair p, q-chunk outer (two 512-wide chunks),
    # k-blocks inner, with `filler` groups interleaved one per (c, j)
    def emit_attn(p, filler):
        expt = {}
        zps = {}

        def emit_scores(c, j):
            qlo = max(512 * c, P * j)
            qhi = 512 * (c + 1)
            w = qhi - qlo
            sps = [None, None]
            for o in range(2):
                sps[o] = pools["pp"].tile([P, w], F32, tag="ps",
                                          name=f"sps_{p}_{c}_{j}_{o}")
                nc.tensor.matmul(
                    sps[o][:],
                    lhsT=kT[o * DH:(o + 1) * DH, p * S + j * P: p * S + (j + 1) * P],
                    rhs=qT[o * DH:(o + 1) * DH, p * S + qlo: p * S + qhi],
                    start=True, stop=True)
            for o in range(2):
                expt[c, j, o] = pools["exp"].tile([P, w], MDT, tag="exp",
                                                  name=f"exp_{p}_{c}_{j}_{o}")
                nc.scalar.activation(
                    expt[c, j, o][:], sps[o][:],
                    mybir.ActivationFunctionType.Exp, scale=0.125)
            if j >= 4 * c:
                # diagonal block at the piece start: keep iff col >= part
                for o in range(2):
                    nc.gpsimd.affine_select(
                        out=expt[c, j, o][:, 0:P], in_=expt[c, j, o][:, 0:P],
                        compare_op=mybir.AluOpType.is_ge,
                        fill=0.0, base=0,
                        pattern=[[1, P]], channel_multiplier=-1)

        def emit_av(c, j):
            jmax = 4 * c + 3
            qlo = max(512 * c, P * j)
            off = qlo - 512 * c
            w = 512 * (c + 1) - qlo
            for o in range(2):
                n = 2 * p + o
                nc.tensor.matmul(
                    zps[c, o][:, off:off + w],
                    lhsT=vaug[:, j * H * VW + n * VW: j * H * VW + (n + 1) * VW],
                    rhs=expt[c, j, o][:],
                    start=(j == 0), stop=(j == jmax))
                del expt[c, j, o]
            if j == jmax:
                for o in range(2):
                    rec = pools["rec"].tile([DH, 512], F32, tag="rec")
                    nc.vector.reciprocal(rec[:], zps[c, o][DH:P, 0:512])
                    nc.vector.tensor_mul(
                        zT[o * DH:(o + 1) * DH, p * S + c * 512: p * S + (c + 1) * 512],
                        zps[c, o][0:DH, 0:512], rec[:])

        gi = 0
        for c in range(2):
            jmax = 4 * c + 3
            for o in range(2):
                zps[c, o] = pools["ppz"].tile([P, 512], F32, tag="psz",
                                              name=f"zps_{p}_{c}_{o}")
            emit_scores(c, 0)
            for j in range(jmax + 1):
                if j + 1 <= jmax:
                    emit_scores(c, j + 1)
                elif c == 0:
                    emit_scores(1, 0)
                emit_av(c, j)
                if gi < len(filler):
                    filler[gi]()
                gi += 1
        for g in filler[gi:]:
            g()

    # ---- prologue: projections for pair 0 (inputs already issued)
    for g in proj_groups(0):
        g()

    for p in range(PAIRS):
        if p + 2 < PAIRS:
            st["wdma"](p + 2)
        elif p + 2 == PAIRS:
            emit_wo_dma()
            prefetch()
        if p + 1 < PAIRS:
            filler = proj_groups(p + 1)
        else:
            # first half of the output projection: s<4 rows of zT are final
            # once this pair's chunk-0 normalize has run, which is before
            # any of the chunk-1 filler slots
            filler = 4 * [lambda: None] + [
                lambda s=s, cb=cb: outproj_group(s, cb)
                for s in range(4) for cb in range(2)]
        emit_attn(p, filler)

    for s in range(4, SB):
        for cb in range(2):
            outproj_group(s, cb)


_CACHED = {}


def build_program(reps=1):
    if reps in _CACHED:
        return _CACHED[reps]
    nc = bacc.Bacc("TRN2", target_bir_lowering=False, debug=False)
    ins = {
        "xT": nc.dram_tensor("xT", [D, S], MDT, kind="ExternalInput").ap(),
        "wq": nc.dram_tensor("wq", [D, D], BF, kind="ExternalInput").ap(),
        "wk": nc.dram_tensor("wk", [D, D], BF, kind="ExternalInput").ap(),
        "wv": nc.dram_tensor("wv", [D, D], MDT, kind="ExternalInput").ap(),
        "wo": nc.dram_tensor("wo", [D, D], MDT, kind="ExternalInput").ap(),
        "bqt": nc.dram_tensor("bqt", [D, 1], F32, kind="ExternalInput").ap(),
        "bkt": nc.dram_tensor("bkt", [D, 1], F32, kind="ExternalInput").ap(),
        "bvb": nc.dram_tensor("bvb", [P, D], F32, kind="ExternalInput").ap(),
        "bob": nc.dram_tensor("bob", [P, D], F32, kind="ExternalInput").ap(),
    }
    out = nc.dram_tensor("out", [S, D], F32, kind="ExternalOutput").ap()
    with tile.TileContext(nc) as tc, ExitStack() as ctx:
        pools = make_pools(tc, ctx)
        st = issue_inputs(tc, pools, ins, first=True)
        for r in range(reps):
            holder = {}

            def prefetch(r=r, holder=holder):
                if r + 1 < reps:
                    holder["st"] = issue_inputs(tc, pools, ins, first=False)

            attention_kernel(tc, out, ins, pools, st, prefetch)
            st = holder.get("st")
    nc.compile()
    _CACHED[reps] = nc
    return nc


def make_in_maps(normalized_resid_pre, W_Q, W_K, W_V, W_O, b_Q, b_K, b_V, b_O):
    x = np.asarray(normalized_resid_pre, np.float32)
    import ml_dtypes
    bf = ml_dtypes.bfloat16

    def pairwise(w):
        # [d_model, head-major] -> pair-contiguous SBUF layout
        # out[pair*128 + p, kc*128 + n] = w[kc*128 + p, pair*128 + n]
        t = np.asarray(w, np.float32).reshape(KC, P, PAIRS, P)
        return np.ascontiguousarray(
            t.transpose(2, 1, 0, 3).reshape(D, D).astype(bf))

    wq_m = pairwise(np.asarray(W_Q, np.float32).transpose(1, 0, 2).reshape(D, D))
    wk_m = pairwise(np.asarray(W_K, np.float32).transpose(1, 0, 2).reshape(D, D))
    wv_m = np.ascontiguousarray(
        np.asarray(W_V, np.float32).transpose(1, 0, 2).reshape(D, D).astype(bf))
    wo_m = np.ascontiguousarray(
        np.asarray(W_O, np.float32).reshape(D, D).astype(bf))
    bq_m = np.asarray(b_Q, np.float32).reshape(D, 1)
    bk_m = np.asarray(b_K, np.float32).reshape(D, 1)
    bv_m = np.ascontiguousarray(np.broadcast_to(
        np.asarray(b_V, np.float32).reshape(1, D), (P, D)))
    bo_m = np.ascontiguousarray(np.broadcast_to(
        np.asarray(b_O, np.float32).reshape(1, D), (P, D)))
    in_maps = []
    for b in range(N_CORES):
        in_maps.append({
            "xT": np.ascontiguousarray(x[b].T).astype(bf),
            "wq": wq_m, "wk": wk_m, "wv": wv_m, "wo": wo_m,
            "bqt": bq_m, "bkt": bk_m, "bvb": bv_m, "bob": bo_m,
        })
    return in_maps


def kernel(**inputs):
    nc = build_program()
    in_maps = make_in_maps(**inputs)
    res = bass_utils.run_bass_kernel_spmd(nc, in_maps, list(range(N_CORES)))
    return np.stack([r["out"] for r in res.results])


# revision 24
# speedup vs baseline: 1.6816x; 1.2814x over previous
"""Causal multi-head attention on 8 Trainium2 NeuronCores.

Problem: x[8,1024,768], 12 heads of d_head=64, causal softmax attention,
output projection. Sharding: data-parallel over batch (8 batch elements ==
8 cores), no collectives.

Per-core layout (zero on-device transposes):
  - host passes x.T [768,1024] bf16 (d_model on partitions)
  - qT/kT [768,1024] head-major rows  (d_head on partitions, seq on free)
  - scoresT[k, q] = kT_blk.T @ qT     (k on partitions, q on free); the two
    heads of a pair run as K=64 matmuls on opposite PE row-group halves
    (auto-derived tile_position) so they can stream concurrently on HW
  - v_aug [1024, 12*128]: per head 64 v columns + 64 ones columns; the AV
    matmul zT_psum = v_aug_blk.T @ exp(scoresT) then yields the softmax
    denominator (replicated) in psum partitions 64..127 for free
  - normalize with DVE reciprocal + tensor_mul (one PSUM operand)
  - out = zT.T @ W_O accumulated over head-pair chunks
Causal structure: only lower-triangular (k <= q) 128-blocks are computed;
the diagonal block is zeroed above the diagonal post-exp via affine_select.
exp runs without max subtraction (scores are O(1) by construction;
exp(-1e5) == 0 in fp32 matches the reference masking).

Schedule: the kernel is engine-balanced rather than phase-sequential.  The
exp traffic (ACT) is the co-bottleneck of the attention loop (~58us vs
~46us of score+AV PE work), while the Q/K/V projections are pure PE work
with ACT idle.  So the projections for pair p+1 are emitted interleaved
into the attention loop of pair p (one 6-matmul projection group per
(chunk, k-block) iteration), and the first half of the output projection
is interleaved into the last pair's attention.  Attention runs q-chunk
outer (two 512-wide q chunks), which needs only 2 PSUM banks for the z
accumulators and leaves 4 for double-buffered scores + 2 for the
projection groups (8 total).  All matmul operands are bf16 (FWL hides the
LDWEIGHTS phase; fp32r stalls ~107ns per matmul on the weight load); PSUM
accumulation is fp32.  End-to-end rel err ~3e-3 vs the fp32 reference
(tolerance 2e-2).

Back-to-back invocations pipeline: x / W_V / first W_Q/W_K pairs / bias
tiles are double-buffered, and each rep issues the next rep's input DMAs
mid-flight on the SP queue (which is idle mid-rep; the ACT/Pool queues
are head-of-line blocked by exp/mask work until the rep ends), so the
next rep's projections start the moment the PE drains.
"""

from contextlib import ExitStack

import numpy as np

import concourse.mybir as mybir
import concourse.tile as tile
from concourse import bacc, bass_utils

F32 = mybir.dt.float32
BF = mybir.dt.bfloat16
MDT = BF

S = 1024        # seq len
D = 768         # d_model
H = 12          # heads
DH = 64         # d_head
P = 128         # partitions
KC = D // P     # 6 k-chunks of d_model
SB = S // P     # 8 seq blocks
PAIRS = H // 2  # 6 head pairs
VW = 2 * DH     # 128: v cols + ones cols per head in v_aug
N_CORES = 8


def make_pools(tc, ctx):
    return {
        "consts": ctx.enter_context(tc.tile_pool(name="consts", bufs=1)),
        "xw": ctx.enter_context(tc.tile_pool(name="xw", bufs=2)),
        "big": ctx.enter_context(tc.tile_pool(name="big", bufs=1)),
        "wqk": ctx.enter_context(tc.tile_pool(name="wqk", bufs=1)),
        "wo": ctx.enter_context(tc.tile_pool(name="wo", bufs=2)),
        "exp": ctx.enter_context(tc.tile_pool(name="exp", bufs=6)),
        "rec": ctx.enter_context(tc.tile_pool(name="rec", bufs=2)),
        "outb": ctx.enter_context(tc.tile_pool(name="outb", bufs=3)),
        "pp": ctx.enter_context(tc.tile_pool(name="ps", bufs=2, space="PSUM")),
        "ppz": ctx.enter_context(tc.tile_pool(name="psz", bufs=2, space="PSUM")),
        "ppj": ctx.enter_context(tc.tile_pool(name="psj", bufs=2, space="PSUM")),
    }


def issue_inputs(tc, pools, ins, first):
    """Allocate the double-buffered input tiles for one rep and issue their
    DMAs.  At kernel start (`first`) the transfers spread across the idle
    ACT/Pool/SP queues; mid-rep prefetch uses only the SP queue (the others
    are head-of-line blocked by the current rep's work)."""
    nc = tc.nc
    st = {}
    st["xt"] = pools["xw"].tile([P, KC * S], MDT, tag="xt", name="xt")
    st["wv"] = pools["xw"].tile([P, KC * D], MDT, tag="wv", name="wv")
    st["wqa"] = pools["wqk"].tile([P, PAIRS * KC * P], BF, tag="wqa", name="wqa")
    st["wka"] = pools["wqk"].tile([P, PAIRS * KC * P], BF, tag="wka", name="wka")
    st["bqt"] = pools["consts"].tile([P, PAIRS], F32, tag="bqt", name="bqt")
    st["bkt"] = pools["consts"].tile([P, PAIRS], F32, tag="bkt", name="bkt")
    st["bvb"] = pools["consts"].tile([P, D], F32, tag="bvb", name="bvb")
    big = pools["big"]
    st["qT"] = big.tile([P, PAIRS * S], MDT, tag="qT", name="qT")
    st["kT"] = big.tile([P, PAIRS * S], MDT, tag="kT", name="kT")
    st["zT"] = big.tile([P, PAIRS * S], MDT, tag="zT", name="zT")
    st["vaug"] = big.tile([P, SB * H * VW], MDT, tag="vaug", name="vaug")
    # ones columns of v_aug (cols 64..127 of each head block): one strided
    # memset over all 96 blocks
    nc.gpsimd.memset(
        st["vaug"][:].rearrange("p (b c) -> p b c", c=VW)[:, :, DH:VW], 1.0)

    # wq/wk whole-tensor (pair-contiguous host layout; one DMA each)
    nc.sync.dma_start(
        st["wqa"][:].rearrange("p (r c) -> p r c", c=KC * P),
        ins["wq"].rearrange("(r p) c -> p r c", p=P))
    nc.sync.dma_start(
        st["wka"][:].rearrange("p (r c) -> p r c", c=KC * P),
        ins["wk"].rearrange("(r p) c -> p r c", p=P))
    if first:
        # x in column-halves (all kc of half 0 first: the first projection
        # groups only need q < 512), spread over the idle ACT/Pool queues
        xq = [nc.scalar, nc.gpsimd]
        for half in range(2):
            for kc in range(KC):
                xq[kc % 2].dma_start(
                    st["xt"][:, kc * S + half * 512: kc * S + (half + 1) * 512],
                    ins["xT"][kc * P:(kc + 1) * P, half * 512:(half + 1) * 512])
        cq = nc.gpsimd
    else:
        nc.sync.dma_start(
            st["xt"][:].rearrange("p (k s) -> p k s", s=S),
            ins["xT"].rearrange("(k p) s -> p k s", p=P))
        cq = nc.sync
    cq.dma_start(st["bqt"][:], ins["bqt"].rearrange("(c p) x -> p (c x)", p=P))
    cq.dma_start(st["bkt"][:], ins["bkt"].rearrange("(c p) x -> p (c x)", p=P))
    cq.dma_start(st["bvb"][:], ins["bvb"][:])
    if first:
        for kc in range(KC):
            cq.dma_start(st["wv"][:, kc * D:(kc + 1) * D],
                         ins["wv"][kc * P:(kc + 1) * P, :])
    else:
        nc.sync.dma_start(
            st["wv"][:].rearrange("p (k d) -> p k d", d=D),
            ins["wv"].rearrange("(k p) d -> p k d", p=P))
    return st


def attention_kernel(tc, out_ap, ins, pools, st, prefetch):
    nc = tc.nc
    qT, kT, zT, vaug = st["qT"], st["kT"], st["zT"], st["vaug"]

    wo = pools["wo"].tile([P, KC * D], MDT, tag="wo")
    bob = pools["wo"].tile([P, D], F32, tag="bob")  # b_O broadcast to 128 rows

    def emit_wo_dma():
        nc.gpsimd.dma_start(bob[:], ins["bob"][:])
        nc.sync.dma_start(wo[:].rearrange("p (k d) -> p k d", d=D),
                          ins["wo"].rearrange("(k p) d -> p k d", p=P))

    # ---- projection groups (6 accumulating matmuls + one DVE drain); sx
    # selects the rep whose tiles are written (the next rep's first
    # projections run inside this rep's last attention pair / output tail)
    def emit_qk_group(sx, p, half, which):
        w_sb = sx["wqa"] if which == 0 else sx["wka"]
        b_sb = sx["bqt"] if which == 0 else sx["bkt"]
        dst = sx["qT"] if which == 0 else sx["kT"]
        ps = pools["ppj"].tile([P, 512], F32, tag="psj")
        for kc in range(KC):
            nc.tensor.matmul(
                ps[:],
                lhsT=w_sb[:, p * KC * P + kc * P: p * KC * P + (kc + 1) * P],
                rhs=sx["xt"][:, kc * S + half * 512: kc * S + (half + 1) * 512],
                start=(kc == 0), stop=(kc == KC - 1))
        nc.vector.tensor_scalar_add(
            dst[:, p * S + half * 512: p * S + (half + 1) * 512],
            ps[:], b_sb[:, p:p + 1])

    def emit_v_group(sx, s, h0, nh):
        # v for heads h0..h0+nh at seq block s: psum [128, nh*64]
        ps = pools["ppj"].tile([P, nh * DH], F32, tag="psj", name=f"vps_{s}_{h0}")
        for kc in range(KC):
            nc.tensor.matmul(
                ps[:],
                lhsT=sx["xt"][:, kc * S + s * P: kc * S + (s + 1) * P],
                rhs=sx["wv"][:, kc * D + h0 * DH: kc * D + (h0 + nh) * DH],
                start=(kc == 0), stop=(kc == KC - 1))
        base = s * H * VW + h0 * VW
        dst3 = sx["vaug"][:, base: base + nh * VW].rearrange(
            "p (n c) -> p n c", c=VW)[:, :, 0:DH]
        src3 = ps[:].rearrange("p (n c) -> p n c", c=DH)
        bv3 = sx["bvb"][:, h0 * DH: (h0 + nh) * DH].rearrange(
            "p (n c) -> p n c", c=DH)
        nc.vector.tensor_add(dst3, src3, bv3)

    def qk_groups(sx, p):
        return [lambda h=half, w=which: emit_qk_group(sx, p, h, w)
                for half in range(2) for which in range(2)]

    def v_groups(sx, h0, nh):
        return [lambda s=s: emit_v_group(sx, s, h0, nh) for s in range(SB)]

    out_tiles = {}

    def outproj_group(s, cb):
        n_cols = 512 if cb == 0 else 256
        outb = out_tiles.get(s)
        if outb is None:
            outb = out_tiles[s] = pools["outb"].tile([P, D], F32, tag="outb",
                                                     name=f"outb_{s}")
        ps = pools["ppj"].tile([P, n_cols], F32, tag="psj", name=f"ops_{s}_{cb}")
        for p in range(PAIRS):
            nc.tensor.matmul(
                ps[:],
                lhsT=zT[:, p * S + s * P: p * S + (s + 1) * P],
                rhs=wo[:, p * D + cb * 512: p * D + cb * 512 + n_cols],
                start=(p == 0), stop=(p == PAIRS - 1))
        nc.vector.tensor_add(outb[:, cb * 512: cb * 512 + n_cols],
                             ps[:],
                             bob[:, cb * 512: cb * 512 + n_cols])
        if cb == 1:
            nc.sync.dma_start(out_ap[s * P:(s + 1) * P, :], outb[:])

    # ---- attention for pair p, q-chunk outer (two 512-wide chunks),
    # k-blocks inner, with `filler` groups interleaved one per (c, j)
    def emit_attn(p, filler):
        expt = {}
        zps = {}

        def emit_scores(c, j):
            qlo = max(512 * c, P * j)
            qhi = 512 * (c + 1)
            w = qhi - qlo
            # both heads' scores in one 2-bank psum tile (head o at cols
            # o*512..), so one exp instruction covers both (the per-instr
            # PSUM access latency on ACT is ~240ns)
            sps = pools["pp"].tile([P, 1024], F32, tag="ps",
                                   name=f"sps_{p}_{c}_{j}")
            for o in range(2):
                nc.tensor.matmul(
                    sps[:, o * 512: o * 512 + w],
                    lhsT=kT[o * DH:(o + 1) * DH, p * S + j * P: p * S + (j + 1) * P],
                    rhs=qT[o * DH:(o + 1) * DH, p * S + qlo: p * S + qhi],
                    start=True, stop=True)
            et = expt[c, j] = pools["exp"].tile([P, 2 * w], MDT, tag="exp",
                                                name=f"exp_{p}_{c}_{j}")
            nc.scalar.activation(
                et[:].rearrange("p (o c) -> p o c", c=w),
                sps[:].rearrange("p (o c) -> p o c", c=512)[:, :, 0:w],
                mybir.ActivationFunctionType.Exp, scale=0.125)
            if j >= 4 * c:
                # diagonal block at the piece start (both heads in one op):
                # keep iff col >= part
                dg = et[:].rearrange("p (o c) -> p o c", c=w)[:, :, 0:P]
                nc.gpsimd.affine_select(
                    out=dg, in_=dg,
                    compare_op=mybir.AluOpType.is_ge,
                    fill=0.0, base=0,
                    pattern=[[0, 2], [1, P]], channel_multiplier=-1)

        def emit_av(c, j):
            jmax = 4 * c + 3
            qlo = max(512 * c, P * j)
            off = qlo - 512 * c
            w = 512 * (c + 1) - qlo
            for o in range(2):
                n = 2 * p + o
                nc.tensor.matmul(
                    zps[c, o][:, off:off + w],
                    lhsT=vaug[:, j * H * VW + n * VW: j * H * VW + (n + 1) * VW],
                    rhs=expt[c, j][:, o * w: (o + 1) * w],
                    start=(j == 0), stop=(j == jmax))
            del expt[c, j]
            if j == jmax:
                for o in range(2):
                    rec = pools["rec"].tile([DH, 512], F32, tag="rec")
                    nc.vector.reciprocal(rec[:], zps[c, o][DH:P, 0:512])
                    nc.vector.tensor_mul(
                        zT[o * DH:(o + 1) * DH, p * S + c * 512: p * S + (c + 1) * 512],
                        zps[c, o][0:DH, 0:512], rec[:])

        # 2-behind software pipeline: scores for (c,j+2) are emitted before
        # the AV of (c,j), so the ACT exp (the per-iteration long pole at
        # ~1.4ns/col vs the PE's 0.83) never stalls the AV matmuls
        seq = [(c, j) for c in range(2) for j in range(4 * c + 4)]
        for c in range(2):
            for o in range(2):
                zps[c, o] = pools["ppz"].tile([P, 512], F32, tag="psz",
                                              name=f"zps_{p}_{c}_{o}")
        emit_scores(*seq[0])
        emit_scores(*seq[1])
        for i, (c, j) in enumerate(seq):
            if i + 2 < len(seq):
                emit_scores(*seq[i + 2])
            if i < len(filler):
                filler[i]()
            emit_av(c, j)
        for g in filler[len(seq):]:
            g()

    # ---- prologue: qk for pair 0, v for heads 0..7 (pairs 0-3), ordered so
    # the half-0 x columns (which arrive first) are consumed first.  A rep
    # whose predecessor already emitted these (cross-rep interleave) skips.
    if not st.get("pre"):
        qk0 = qk_groups(st, 0)
        vq = v_groups(st, 0, 8)
        for g in [qk0[0], qk0[1]] + vq[0:4] + [qk0[2], qk0[3]] + vq[4:8]:
            g()

    st_next = None
    for p in range(PAIRS):
        if p + 2 == PAIRS:
            emit_wo_dma()
            st_next = prefetch()
        if p == 0:
            filler = qk_groups(st, 1) + v_groups(st, 8, 4)
        elif p + 1 < PAIRS:
            filler = qk_groups(st, p + 1)
        else:
            # c0 slots: the next rep's first q/k projections (its inputs
            # were prefetched at p==4).  c1 slots: the first half of the
            # output projection — s<4 rows of zT are final once this pair's
            # chunk-0 normalize has run.
            nqk = qk_groups(st_next, 0) if st_next else 4 * [lambda: None]
            filler = nqk + [
                lambda s=s, cb=cb: outproj_group(s, cb)
                for s in range(4) for cb in range(2)]
        emit_attn(p, filler)

    # tail: remaining output projection, interleaved with the next rep's
    # v projection for heads 0..7 (pure PE filler for the zT/DVE stalls)
    nvq = v_groups(st_next, 0, 8) if st_next else 8 * [lambda: None]
    ti = 0
    for s in range(4, SB):
        for cb in range(2):
            outproj_group(s, cb)
            if ti < 8:
                nvq[ti]()
            ti += 1
    if st_next is not None:
        st_next["pre"] = True
    return st_next


_CACHED = {}


def build_program(reps=1):
    if reps in _CACHED:
        return _CACHED[reps]
    nc = bacc.Bacc("TRN2", target_bir_lowering=False, debug=False)
    ins = {
        "xT": nc.dram_tensor("xT", [D, S], MDT, kind="ExternalInput").ap(),
        "wq": nc.dram_tensor("wq", [D, D], BF, kind="ExternalInput").ap(),
        "wk": nc.dram_tensor("wk", [D, D], BF, kind="ExternalInput").ap(),
        "wv": nc.dram_tensor("wv", [D, D], MDT, kind="ExternalInput").ap(),
        "wo": nc.dram_tensor("wo", [D, D], MDT, kind="ExternalInput").ap(),
        "bqt": nc.dram_tensor("bqt", [D, 1], F32, kind="ExternalInput").ap(),
        "bkt": nc.dram_tensor("bkt", [D, 1], F32, kind="ExternalInput").ap(),
        "bvb": nc.dram_tensor("bvb", [P, D], F32, kind="ExternalInput").ap(),
        "bob": nc.dram_tensor("bob", [P, D], F32, kind="ExternalInput").ap(),
    }
    out = nc.dram_tensor("out", [S, D], F32, kind="ExternalOutput").ap()
    with tile.TileContext(nc) as tc, ExitStack() as ctx:
        pools = make_pools(tc, ctx)
        st = issue_inputs(tc, pools, ins, first=True)
        for r in range(reps):

            def prefetch(r=r):
                if r + 1 < reps:
                    return issue_inputs(tc, pools, ins, first=False)
                return None

            st = attention_kernel(tc, out, ins, pools, st, prefetch)
    nc.compile()
    _CACHED[reps] = nc
    return nc


def make_in_maps(normalized_resid_pre, W_Q, W_K, W_V, W_O, b_Q, b_K, b_V, b_O):
    x = np.asarray(normalized_resid_pre, np.float32)
    import ml_dtypes
    bf = ml_dtypes.bfloat16

    def pairwise(w):
        # [d_model, head-major] -> pair-contiguous SBUF layout
        # out[pair*128 + p, kc*128 + n] = w[kc*128 + p, pair*128 + n]
        t = np.asarray(w, np.float32).reshape(KC, P, PAIRS, P)
        return np.ascontiguousarray(
            t.transpose(2, 1, 0, 3).reshape(D, D).astype(bf))

    wq_m = pairwise(np.asarray(W_Q, np.float32).transpose(1, 0, 2).reshape(D, D))
    wk_m = pairwise(np.asarray(W_K, np.float32).transpose(1, 0, 2).reshape(D, D))
    wv_m = np.ascontiguousarray(
        np.asarray(W_V, np.float32).transpose(1, 0, 2).reshape(D, D).astype(bf))
    wo_m = np.ascontiguousarray(
        np.asarray(W_O, np.float32).reshape(D, D).astype(bf))
    bq_m = np.asarray(b_Q, np.float32).reshape(D, 1)
    bk_m = np.asarray(b_K, np.float32).reshape(D, 1)
    bv_m = np.ascontiguousarray(np.broadcast_to(
        np.asarray(b_V, np.float32).reshape(1, D), (P, D)))
    bo_m = np.ascontiguousarray(np.broadcast_to(
        np.asarray(b_O, np.float32).reshape(1, D), (P, D)))
    in_maps = []
    for b in range(N_CORES):
        in_maps.append({
            "xT": np.ascontiguousarray(x[b].T).astype(bf),
            "wq": wq_m, "wk": wk_m, "wv": wv_m, "wo": wo_m,
            "bqt": bq_m, "bkt": bk_m, "bvb": bv_m, "bob": bo_m,
        })
    return in_maps


def kernel(**inputs):
    nc = build_program()
    in_maps = make_in_maps(**inputs)
    res = bass_utils.run_bass_kernel_spmd(nc, in_maps, list(range(N_CORES)))
    return np.stack([r["out"] for r in res.results])
